# revision 28
# baseline (speedup 1.0000x reference)
"""Trainium2 Bass kernel for nn_Attention_88441966559243.

Attention with additive bias [B,N,N] and per-key bool mask, fp32.
  B=2, N=2048, QD=1024, HEADS=16, DIM_HEAD=64.

Sharding: 8 cores = (batch b = core//4) x (query slice q0 = (core%4)*512).
Each core computes out[b, q0:q0+512, :] completely on-device; the host gather
is a pure concatenation. No collectives.

v2 design (vs the PE-transpose + 4x-ident-inject baseline):
  - x and the core's bias slice are uploaded pre-transposed (host layout
    change only), so no PE transposes on device.
  - EB = exp(biasT + maskneg) is precomputed once per core on ACT (exp(-30000)
    underflows to exactly 0, folding the key mask in); the softmax numerator
    is then e = exp(sim*scale) * EB with the multiply on DVE in bf16.  This
    removes the old per-tile PE identity-matmul bias injection entirely.
  - k^T slabs stay SBUF-resident: each head-pair's slab is computed on PE
    interleaved with the previous pair's attention kc-loop, so there is no
    k DRAM round-trip and the slab matmuls hide under the ACT exp stream
    (the critical path, ~1.15us per 128x1024 exp tile).
  - v' = [v | 1] in bf16 goes through DRAM once (quad-head-packed loads,
    520B rows); ACT does only exp; PSUM evacuations ride on DVE.
"""
import sys
for _p in ("/opt/trn_rl_repo", "/root/.axon_site/_ro/trn_rl_repo"):
    if _p not in sys.path:
        sys.path.insert(0, _p)

import numpy as np

import concourse.bass as bass
import concourse.mybir as mybir
from concourse import bacc
from concourse.tile import TileContext
from concourse.bass_utils import run_bass_kernel_spmd

F = 1024          # feature dim (QD == INNER)
NK = 2048         # keys (full sequence)
Q = 512           # queries per core
H = 16            # heads
D = 64            # head dim
DV = 65           # head dim + ones column
SCALE = D ** -0.5
MASK_NEG = -30000.0

FC = F // 128      # 8 feature chunks
KC = NK // 128     # 16 key chunks
HP = H // 2        # 8 head pairs
NB = NK // 512     # 4 key 512-blocks

f32 = mybir.dt.float32
fr = mybir.dt.float32r
bf16 = mybir.dt.bfloat16
AF = mybir.ActivationFunctionType

AV_LAG = 2         # av matmuls trail the exp/mul producers by this many kc


def build_nc(niter: int = 1):
    nc = bacc.Bacc(None, target_bir_lowering=False)

    xT_in = nc.dram_tensor("xT_in", [F, NK], fr, kind="ExternalInput")
    biasT_in = nc.dram_tensor("biasT_in", [NK, Q], f32, kind="ExternalInput")
    maskneg_in = nc.dram_tensor("maskneg_in", [128, KC], f32, kind="ExternalInput")
    wq_in = nc.dram_tensor("wq_in", [F, F], fr, kind="ExternalInput")
    wkv_in = nc.dram_tensor("wkv_in", [F, 2 * F], fr, kind="ExternalInput")
    wo_in = nc.dram_tensor("wo_in", [F, F], bf16, kind="ExternalInput")
    bo_in = nc.dram_tensor("bo_in", [1, F], fr, kind="ExternalInput")
    out_t = nc.dram_tensor("out_t", [Q, F], f32, kind="ExternalOutput")

    with TileContext(nc) as tc:
        with (
            tc.tile_pool(name="const", bufs=1) as constp,
            tc.tile_pool(name="dram", bufs=1, space="DRAM") as dramp,
            tc.tile_pool(name="ps2", bufs=2, space="PSUM") as ps2p,   # [128,1024]
            tc.tile_pool(name="psk", bufs=2, space="PSUM") as pskp,   # [128,512]
            tc.tile_pool(name="psu", bufs=2, space="PSUM") as psup,   # [DV,512]
        ):
            ones_f = constp.tile([128, 128], f32)
            nc.vector.memset(ones_f[:, :], 1.0)
            ones_r = constp.tile([128, 128], fr)
            nc.scalar.copy(ones_r[:, :], ones_f[:, :])
            masksb = constp.tile([128, KC], f32)
            nc.sync.dma_start(masksb[:, :], maskneg_in[:, :])

            vprime = dramp.tile([NK, H * DV], bf16)     # v' (keys-major)

            def body(_iv=None):
              with (
                  tc.tile_pool(name="oMp", bufs=1) as oMp,
                  tc.tile_pool(name="wop", bufs=1) as wop,
              ):
                # out^T head-pair tiles (bf16), written in-place by C's ot
                # DMAs; wo (bf16) loaded near the end of C.  Stacking heads
                # 2i/2i+1 on partitions 0:64/64:128 makes D K=128 full-rate.
                oM = [oMp.tile([128, Q], bf16, tag=f"oM{i}", name=f"oM{i}")
                      for i in range(H // 2)]
                wo = [wop.tile([128, F], bf16, tag=f"wo{i}", name=f"wo{i}")
                      for i in range(H // 2)]
                with (
                    tc.tile_pool(name="EBp", bufs=1) as EBp,
                    tc.tile_pool(name="qTp", bufs=1) as qTp,
                ):
                    EB = [EBp.tile([128, 2 * Q], bf16, tag=f"EB{i}",
                                   name=f"EB{i}") for i in range(KC)]
                    qT = [qTp.tile([128, Q], fr, tag=f"qT{i}", name=f"qT{i}")
                          for i in range(FC)]

                    with tc.tile_pool(name="xTp", bufs=1) as xTp:
                        # xT as 32 column tiles, loaded col-block-major AFTER
                        # xqT/wq/wv so A2 starts at ~6MB and A4 streams.
                        xTc = [xTp.tile([128, 512], fr, tag=f"xT{i}",
                                        name=f"xT{i}") for i in range(4 * FC)]

                        def xt(fc, cb):
                            return xTc[fc * 4 + cb]

                        # ---- A: q and v' projections (weights scoped) ----
                        # host rolls the key axis by -q0 per core, so the
                        # query block is always xT columns 0:512 (cb 0)
                        with tc.tile_pool(name="wload", bufs=8) as wlp:
                            wq = [wlp.tile([128, F], fr, tag="w", name="w")
                                  for _ in range(FC)]
                            for fc in range(FC):
                                nc.sync.dma_start(
                                    wq[fc][:, :],
                                    wq_in[fc * 128:(fc + 1) * 128, :])
                            for fc in range(FC):
                                nc.sync.dma_start(
                                    xt(fc, 0)[:, :],
                                    xT_in[fc * 128:(fc + 1) * 128, 0:512])
                            wv = [wlp.tile([128, F], fr, tag="w", name="w")
                                  for _ in range(FC)]
                            for fc in range(FC):
                                nc.sync.dma_start(
                                    wv[fc][:, :],
                                    wkv_in[fc * 128:(fc + 1) * 128, F:2 * F])
                            for cb in range(1, 4):
                                for fc in range(FC):
                                    nc.sync.dma_start(
                                        xt(fc, cb)[:, :],
                                        xT_in[fc * 128:(fc + 1) * 128,
                                              cb * 512:(cb + 1) * 512])

                            # A2: qT[m] = Wq[:,m]^T @ xT[:, 0:Q]  (unscaled;
                            # SCALE rides on the exp activation's scale)
                            for m in range(FC):
                                ps = pskp.tile([128, 512], f32, name="psk")
                                for fc in range(FC):
                                    nc.tensor.matmul(
                                        ps[:, :],
                                        wq[fc][:, m * 128:(m + 1) * 128],
                                        xt(fc, 0)[:, :],
                                        start=(fc == 0), stop=(fc == FC - 1))
                                nc.vector.tensor_copy(qT[m][:, :], ps[:, :])

                            # ---- B: EB = exp(biasT + maskneg); biasT loads
                            # ride the ACT hwdge queue so the SP queue's A4
                            # v' stores aren't blocked behind them ----
                            with tc.tile_pool(name="bT", bufs=4) as bTp:
                                for kc in range(KC):
                                    bT = bTp.tile([128, Q], f32, name="bT")
                                    nc.scalar.dma_start(
                                        bT[:, :],
                                        biasT_in[kc * 128:(kc + 1) * 128, :])
                                    for half in range(2):
                                        nc.scalar.activation(
                                            EB[kc][:, half * Q:(half + 1) * Q],
                                            bT[:, :], AF.Exp,
                                            bias=masksb[:, kc:kc + 1],
                                            scale=1.0)

                            # A4: v' = [x @ Wv | 1] -> DRAM bf16, keys-major
                            with tc.tile_pool(name="vstg", bufs=6) as vstgp:
                                for kc in range(KC):
                                    vst = vstgp.tile([128, H * DV], bf16,
                                                     name="vst")
                                    nc.vector.memset(
                                        vst[:, :].rearrange(
                                            "p (h x) -> p h x",
                                            x=DV)[:, :, D:DV], 1.0)
                                    ps = ps2p.tile([128, 1024], f32,
                                                   name="ps2")
                                    for half in range(2):
                                        for fc in range(FC):
                                            nc.tensor.matmul(
                                                ps[:, half * 512:
                                                   (half + 1) * 512],
                                                xt(fc, kc // 4)[
                                                    :, (kc % 4) * 128:
                                                    (kc % 4 + 1) * 128],
                                                wv[fc][:, half * 512:
                                                       (half + 1) * 512],
                                                start=(fc == 0),
                                                stop=(fc == FC - 1))
                                    nc.vector.tensor_copy(
                                        vst[:, :].rearrange(
                                            "p (h x) -> p h x",
                                            x=DV)[:, :, 0:D],
                                        ps[:, :].rearrange(
                                            "p (h d) -> p h d", d=D))
                                    nc.sync.dma_start(
                                        vprime[kc * 128:(kc + 1) * 128, :],
                                        vst[:, :])

                        # ---- C: attention; k-slab hp+1 on PE under the
                        # hp kc-loop; v' quad loads (520B rows) ----
                        with (
                            tc.tile_pool(name="wkhp", bufs=2) as wkhpp,
                            tc.tile_pool(name="kst", bufs=2) as kstp,
                            tc.tile_pool(name="vph", bufs=2) as vphp,
                            tc.tile_pool(name="eraw", bufs=3) as erawp,
                            tc.tile_pool(name="et", bufs=5) as ep,
                            tc.tile_pool(name="dsb", bufs=2) as dsbp,
                            tc.tile_pool(name="rrep", bufs=2) as rrepp,
                            tc.tile_pool(name="otst", bufs=2) as otstp,
                        ):
                            def load_wk(hp):
                                wkhp = wkhpp.tile([128, F], fr, name="wkhp")
                                nc.sync.dma_start(
                                    wkhp[:, :].rearrange(
                                        "p (fc c) -> p fc c", c=128),
                                    wkv_in[0:F, hp * 128:(hp + 1) * 128]
                                    .rearrange("(fc p) c -> p fc c", p=128))
                                return wkhp

                            def load_vquad(hq):
                                vph = vphp.tile([128, KC * 4 * DV], bf16,
                                                name="vph")
                                nc.sync.dma_start(
                                    vph[:, :].rearrange(
                                        "p (kc d) -> p kc d", d=4 * DV),
                                    vprime[:, 4 * hq * DV:(4 * hq + 4) * DV]
                                    .rearrange("(kc p) d -> p kc d", p=128))
                                return vph

                            def a3_slab(wkhp):
                                """k-slab matmuls as closures, 1 per call."""
                                kst = kstp.tile([128, NK], fr, name="kst")
                                state = {}

                                def mk(nb, fc):
                                    def run():
                                        if fc == 0:
                                            state[nb] = pskp.tile(
                                                [128, 512], f32, name="psk")
                                        nc.tensor.matmul(
                                            state[nb][:, :],
                                            wkhp[:, fc * 128:(fc + 1) * 128],
                                            xt(fc, nb)[:, :],
                                            start=(fc == 0),
                                            stop=(fc == FC - 1))
                                        if fc == FC - 1:
                                            nc.vector.tensor_copy(
                                                kst[:, nb * 512:
                                                    (nb + 1) * 512],
                                                state.pop(nb)[:, :])
                                    return run

                                return kst, [mk(nb, fc) for nb in range(NB)
                                             for fc in range(FC)]

                            def emit_denoms(hp0, psU0):
                                """Softmax normalize + oM write for pair hp0;
                                deferred into the next pair's kc loop so the
                                PE sim stream isn't interrupted."""
                                for sub in range(2):
                                    Dsb = dsbp.tile([DV, 512], fr,
                                                    name="Dsb")
                                    nc.vector.tensor_copy(
                                        Dsb[64:65, :], psU0[sub][64:65, :])
                                    psR = pskp.tile([128, 512], f32,
                                                    name="psk")
                                    nc.tensor.matmul(psR[0:64, :],
                                                     ones_r[64:65, 0:64],
                                                     Dsb[64:65, :],
                                                     start=True, stop=True)
                                    rrep = rrepp.tile([64, 512], f32,
                                                      name="rrep")
                                    nc.vector.reciprocal_approx_fast(
                                        out=rrep[:, :], in_=psR[0:64, :])
                                    ot = otstp.tile([64, Q], bf16, name="ot")
                                    nc.vector.tensor_mul(ot[:, :],
                                                         psU0[sub][0:64, :],
                                                         rrep[:, :])
                                    nc.sync.dma_start(
                                        oM[hp0][sub * 64:(sub + 1) * 64, :],
                                        ot[:, :])

                            wk0 = load_wk(0)
                            kst_cur, ops0 = a3_slab(wk0)
                            for op in ops0:
                                op()
                            wk_next = load_wk(1)
                            vph_cur = load_vquad(0)
                            prev = None

                            for hp in range(HP):
                                if hp == 6:
                                    for i in range(H // 2):
                                        nc.sync.dma_start(
                                            wo[i][:, :],
                                            wo_in[i * 128:(i + 1) * 128, :])
                                if hp % 2 == 1 and hp + 1 < HP:
                                    vph_next = load_vquad((hp + 1) // 2)
                                if hp + 1 < HP:
                                    kst_next, a3_ops = a3_slab(wk_next)
                                    if hp + 2 < HP:
                                        wk_next = load_wk(hp + 2)
                                else:
                                    kst_next, a3_ops = None, []
                                psU = [psup.tile([DV, 512], f32, name="psu")
                                       for _ in range(2)]
                                pending = []

                                def drain_av(upto):
                                    while pending and pending[0][0] <= upto:
                                        kc0, eT = pending.pop(0)
                                        for sub in range(2):
                                            hq_off = (2 * hp + sub) % 4
                                            nc.tensor.matmul(
                                                psU[sub][:, :],
                                                vph_cur[:,
                                                        kc0 * 4 * DV
                                                        + hq_off * DV:
                                                        kc0 * 4 * DV
                                                        + (hq_off + 1) * DV],
                                                eT[:, sub * Q:(sub + 1) * Q],
                                                start=(kc0 == 0),
                                                stop=(kc0 == KC - 1))

                                for kc in range(KC):
                                    ps = ps2p.tile([128, 1024], f32,
                                                   name="ps2")
                                    for sub in range(2):
                                        po = sub * 64
                                        nc.tensor.matmul(
                                            ps[:, sub * Q:(sub + 1) * Q],
                                            kst_cur[po:po + 64,
                                                    kc * 128:(kc + 1) * 128],
                                            qT[hp][po:po + 64, :],
                                            start=True, stop=True)
                                    if kc == 1 and prev is not None:
                                        emit_denoms(*prev)
                                    # 2 slab matmuls for hp+1 per kc chunk
                                    for _ in range(2):
                                        if a3_ops:
                                            a3_ops.pop(0)()
                                    eRaw = erawp.tile([128, 1024], bf16,
                                                      name="eRaw")
                                    nc.scalar.activation(
                                        eRaw[:, :], ps[:, :], AF.Exp,
                                        scale=SCALE)
                                    eT = ep.tile([128, 1024], bf16,
                                                 name="eT")
                                    nc.vector.tensor_mul(eT[:, :],
                                                         eRaw[:, :],
                                                         EB[kc][:, :])
                                    pending.append((kc, eT))
                                    drain_av(kc - AV_LAG)
                                for op in a3_ops:
                                    op()
                                drain_av(KC)
                                prev = (hp, psU)
                                kst_cur = kst_next
                                if hp % 2 == 1 and hp + 1 < HP:
                                    vph_cur = vph_next
                            emit_denoms(*prev)

                # ======== stage D ========
                with (
                    tc.tile_pool(name="fin", bufs=3) as finp,
                    tc.tile_pool(name="bop", bufs=1) as bop,
                ):
                    bo_sb = bop.tile([1, F], fr, name="bo_sb")
                    nc.sync.dma_start(bo_sb[:, :], bo_in[:, :])
                    bo_rep = bop.tile([128, F], f32, name="bo_rep")
                    for nb2 in range(2):
                        ps = pskp.tile([128, 512], f32, name="psk")
                        nc.tensor.matmul(ps[:, :], ones_r[0:1, 0:128],
                                         bo_sb[0:1, nb2 * 512:(nb2 + 1) * 512],
                                         start=True, stop=True)
                        nc.vector.tensor_copy(
                            bo_rep[:, nb2 * 512:(nb2 + 1) * 512], ps[:, :])
                    for mc in range(4):
                        for nb2 in range(2):
                            psF = pskp.tile([128, 512], f32, name="psk")
                            for h in range(H // 2):
                                nc.tensor.matmul(
                                    psF[:, :],
                                    oM[h][:, mc * 128:(mc + 1) * 128],
                                    wo[h][:, nb2 * 512:(nb2 + 1) * 512],
                                    start=(h == 0), stop=(h == H // 2 - 1))
                            fin = finp.tile([128, 512], f32, name="fin")
                            nc.vector.tensor_add(
                                fin[:, :], psF[:, :],
                                bo_rep[:, nb2 * 512:(nb2 + 1) * 512])
                            nc.sync.dma_start(
                                out_t[mc * 128:(mc + 1) * 128,
                                      nb2 * 512:(nb2 + 1) * 512],
                                fin[:, :])

            if niter == 1:
                body()
            else:
                with tc.For_i(0, niter, 1) as iv:
                    body(iv)

    nc.finalize()
    return nc


_nc_cache = {}


def _get_nc(niter=1):
    if niter not in _nc_cache:
        _nc_cache[niter] = build_nc(niter)
    return _nc_cache[niter]


def make_in_maps(x, bias, mask, Wq, Wkv, Wo, bo):
    x = np.asarray(x, dtype=np.float32)
    bias = np.asarray(bias, dtype=np.float32)
    mask = np.asarray(mask)
    in_maps = []
    for c in range(8):
        b, qi = c // 4, c % 4
        q0 = qi * Q
        # roll the KEY axis by -q0 (softmax/attention are permutation-
        # invariant over keys when bias/mask roll consistently); the core's
        # query block is then always xT columns 0:512
        maskneg = np.roll(
            np.where(mask[b], 0.0, MASK_NEG).astype(np.float32), -q0)
        in_maps.append({
            "xT_in": np.ascontiguousarray(np.roll(x[b].T, -q0, axis=1)),
            "biasT_in": np.ascontiguousarray(
                np.roll(bias[b, q0:q0 + Q].T, -q0, axis=0)),
            "maskneg_in": np.ascontiguousarray(maskneg.reshape(KC, 128).T),
            "wq_in": np.ascontiguousarray(np.asarray(Wq, dtype=np.float32)),
            "wkv_in": np.ascontiguousarray(np.asarray(Wkv, dtype=np.float32)),
            "wo_in": np.ascontiguousarray(
                np.asarray(Wo).astype(mybir.dt.np(bf16))),
            "bo_in": np.ascontiguousarray(
                np.asarray(bo, dtype=np.float32).reshape(1, F)),
        })
    return in_maps


class _CachedRunner:
    """Jit the NEFF-backed executable once; repeat kernel() calls then skip
    the ~40s relower/recompile and run in ~0.1s."""

    def __init__(self, nc, n_cores=8):
        import jax
        from jax.sharding import Mesh, PartitionSpec
        from jax.experimental.shard_map import shard_map
        from concourse.bass2jax import (_bass_exec_p, install_neuronx_cc_hook,
                                        partition_id_tensor)
        install_neuronx_cc_hook()
        self.jax = jax
        self.n_cores = n_cores
        pname = nc.partition_id_tensor.name if nc.partition_id_tensor else None
        in_names, out_names, out_avals, zeros = [], [], [], []
        for alloc in nc.m.functions[0].allocations:
            if not isinstance(alloc, mybir.MemoryLocationSet):
                continue
            name = alloc.memorylocations[0].name
            if alloc.kind == "ExternalInput":
                if name != pname:
                    in_names.append(name)
            elif alloc.kind == "ExternalOutput":
                out_names.append(name)
                shape = tuple(alloc.tensor_shape)
                dt_np = mybir.dt.np(alloc.dtype)
                out_avals.append(jax.core.ShapedArray(shape, dt_np))
                zeros.append(np.zeros(shape, dt_np))
        self.in_names, self.out_names = in_names, out_names
        self.out_avals, self.zeros = out_avals, zeros
        all_names = in_names + out_names + ([pname] if pname else [])

        def _body(*args):
            ops = list(args)
            if pname is not None:
                ops.append(partition_id_tensor())
            return tuple(_bass_exec_p.bind(
                *ops, out_avals=tuple(out_avals), in_names=tuple(all_names),
                out_names=tuple(out_names), lowering_input_output_aliases=(),
                sim_require_finite=True, sim_require_nnan=True, nc=nc))

        mesh = Mesh(np.asarray(jax.devices()[:n_cores]), ("core",))
        spec_in = (PartitionSpec("core"),) * (len(in_names) + len(out_names))
        spec_out = (PartitionSpec("core"),) * len(out_names)
        self.fn = jax.jit(shard_map(_body, mesh=mesh, in_specs=spec_in,
                                    out_specs=spec_out, check_rep=False),
                          keep_unused=True)

    def run(self, in_maps):
        n = self.n_cores
        args = [np.concatenate([np.asarray(in_maps[c][k]) for c in range(n)], axis=0)
                for k in self.in_names]
        args += [np.zeros((n * z.shape[0], *z.shape[1:]), z.dtype)
                 for z in self.zeros]
        outs = self.fn(*args)
        self.jax.block_until_ready(outs)
        return [{k: np.asarray(outs[i]).reshape(n, *self.out_avals[i].shape)[c]
                 for i, k in enumerate(self.out_names)} for c in range(n)]


_runner_cache = {}


def kernel(x, bias, mask, Wq, Wkv, Wo, bo):
    in_maps = make_in_maps(x, bias, mask, Wq, Wkv, Wo, bo)
    try:
        if "r" not in _runner_cache:
            _runner_cache["r"] = _CachedRunner(_get_nc(1))
        results = _runner_cache["r"].run(in_maps)
    except Exception:
        _runner_cache.pop("r", None)
        res = run_bass_kernel_spmd(_get_nc(1), in_maps, core_ids=list(range(8)))
        results = res.results
    out = np.empty((2, NK, F), dtype=np.float32)
    for c in range(8):
        b, qi = c // 4, c % 4
        out[b, qi * Q:(qi + 1) * Q] = results[c]["out_t"]
    return out


# revision 31
# speedup vs baseline: 1.0026x; 1.0026x over previous
"""Trainium2 Bass kernel for nn_Attention_88441966559243.

Attention with additive bias [B,N,N] and per-key bool mask, fp32.
  B=2, N=2048, QD=1024, HEADS=16, DIM_HEAD=64.

Sharding: 8 cores = (batch b = core//4) x (query slice q0 = (core%4)*512).
Each core computes out[b, q0:q0+512, :] completely on-device; the host gather
is a pure concatenation. No collectives.

v2 design (vs the PE-transpose + 4x-ident-inject baseline):
  - x and the core's bias slice are uploaded pre-transposed (host layout
    change only), so no PE transposes on device.
  - EB = exp(biasT + maskneg) is precomputed once per core on ACT (exp(-30000)
    underflows to exactly 0, folding the key mask in); the softmax numerator
    is then e = exp(sim*scale) * EB with the multiply on DVE in bf16.  This
    removes the old per-tile PE identity-matmul bias injection entirely.
  - k^T slabs stay SBUF-resident: each head-pair's slab is computed on PE
    interleaved with the previous pair's attention kc-loop, so there is no
    k DRAM round-trip and the slab matmuls hide under the ACT exp stream
    (the critical path, ~1.15us per 128x1024 exp tile).
  - v' = [v | 1] in bf16 goes through DRAM once (quad-head-packed loads,
    520B rows); ACT does only exp; PSUM evacuations ride on DVE.
"""
import sys
for _p in ("/opt/trn_rl_repo", "/root/.axon_site/_ro/trn_rl_repo"):
    if _p not in sys.path:
        sys.path.insert(0, _p)

import numpy as np

import concourse.bass as bass
import concourse.mybir as mybir
from concourse import bacc
from concourse.tile import TileContext
from concourse.bass_utils import run_bass_kernel_spmd

F = 1024          # feature dim (QD == INNER)
NK = 2048         # keys (full sequence)
Q = 512           # queries per core
H = 16            # heads
D = 64            # head dim
DV = 65           # head dim + ones column
SCALE = D ** -0.5
MASK_NEG = -30000.0

FC = F // 128      # 8 feature chunks
KC = NK // 128     # 16 key chunks
HP = H // 2        # 8 head pairs
NB = NK // 512     # 4 key 512-blocks

f32 = mybir.dt.float32
fr = mybir.dt.float32r
bf16 = mybir.dt.bfloat16
AF = mybir.ActivationFunctionType

AV_LAG = 4         # av matmuls trail the exp/mul producers by this many kc


def build_nc(niter: int = 1):
    nc = bacc.Bacc(None, target_bir_lowering=False)

    xT_in = nc.dram_tensor("xT_in", [F, NK], fr, kind="ExternalInput")
    biasT_in = nc.dram_tensor("biasT_in", [NK, Q], f32, kind="ExternalInput")
    maskneg_in = nc.dram_tensor("maskneg_in", [128, KC], f32, kind="ExternalInput")
    wq_in = nc.dram_tensor("wq_in", [F, F], fr, kind="ExternalInput")
    wkv_in = nc.dram_tensor("wkv_in", [F, 2 * F], fr, kind="ExternalInput")
    wo_in = nc.dram_tensor("wo_in", [F, F], bf16, kind="ExternalInput")
    bo_in = nc.dram_tensor("bo_in", [1, F], fr, kind="ExternalInput")
    out_t = nc.dram_tensor("out_t", [Q, F], f32, kind="ExternalOutput")

    with TileContext(nc) as tc:
        with (
            tc.tile_pool(name="const", bufs=1) as constp,
            tc.tile_pool(name="dram", bufs=1, space="DRAM") as dramp,
            tc.tile_pool(name="ps2", bufs=2, space="PSUM") as ps2p,   # [128,1024]
            tc.tile_pool(name="psk", bufs=2, space="PSUM") as pskp,   # [128,512]
            tc.tile_pool(name="psu", bufs=2, space="PSUM") as psup,   # [DV,512]
        ):
            ones_f = constp.tile([128, 128], f32)
            nc.vector.memset(ones_f[:, :], 1.0)
            ones_r = constp.tile([128, 128], fr)
            nc.scalar.copy(ones_r[:, :], ones_f[:, :])
            masksb = constp.tile([128, KC], f32)
            nc.sync.dma_start(masksb[:, :], maskneg_in[:, :])

            vprime = dramp.tile([NK, H * DV], bf16)     # v' (keys-major)

            def body(_iv=None):
              with (
                  tc.tile_pool(name="oMp", bufs=1) as oMp,
                  tc.tile_pool(name="wop", bufs=1) as wop,
              ):
                # out^T head-pair tiles (bf16), written in-place by C's ot
                # DMAs; wo (bf16) loaded near the end of C.  Stacking heads
                # 2i/2i+1 on partitions 0:64/64:128 makes D K=128 full-rate.
                oM = [oMp.tile([128, Q], bf16, tag=f"oM{i}", name=f"oM{i}")
                      for i in range(H // 2)]
                wo = [wop.tile([128, F], bf16, tag=f"wo{i}", name=f"wo{i}")
                      for i in range(H // 2)]
                with (
                    tc.tile_pool(name="EBp", bufs=1) as EBp,
                    tc.tile_pool(name="qTp", bufs=1) as qTp,
                ):
                    EB = [EBp.tile([128, 2 * Q], bf16, tag=f"EB{i}",
                                   name=f"EB{i}") for i in range(KC)]
                    qT = [qTp.tile([128, Q], fr, tag=f"qT{i}", name=f"qT{i}")
                          for i in range(FC)]

                    with tc.tile_pool(name="xTp", bufs=1) as xTp:
                        # xT as 32 column tiles, loaded col-block-major AFTER
                        # xqT/wq/wv so A2 starts at ~6MB and A4 streams.
                        xTc = [xTp.tile([128, 512], fr, tag=f"xT{i}",
                                        name=f"xT{i}") for i in range(4 * FC)]

                        def xt(fc, cb):
                            return xTc[fc * 4 + cb]

                        # ---- A: q and v' projections (weights scoped) ----
                        # host rolls the key axis by -q0 per core, so the
                        # query block is always xT columns 0:512 (cb 0)
                        with tc.tile_pool(name="wload", bufs=8) as wlp:
                            wq = [wlp.tile([128, F], fr, tag="w", name="w")
                                  for _ in range(FC)]
                            for fc in range(FC):
                                nc.sync.dma_start(
                                    wq[fc][:, :],
                                    wq_in[fc * 128:(fc + 1) * 128, :])
                            for fc in range(FC):
                                nc.sync.dma_start(
                                    xt(fc, 0)[:, :],
                                    xT_in[fc * 128:(fc + 1) * 128, 0:512])
                            wv = [wlp.tile([128, F], fr, tag="w", name="w")
                                  for _ in range(FC)]
                            for fc in range(FC):
                                nc.sync.dma_start(
                                    wv[fc][:, :],
                                    wkv_in[fc * 128:(fc + 1) * 128, F:2 * F])
                            for cb in range(1, 4):
                                for fc in range(FC):
                                    nc.sync.dma_start(
                                        xt(fc, cb)[:, :],
                                        xT_in[fc * 128:(fc + 1) * 128,
                                              cb * 512:(cb + 1) * 512])

                            # A2: qT[m] = Wq[:,m]^T @ xT[:, 0:Q]  (unscaled;
                            # SCALE rides on the exp activation's scale)
                            for m in range(FC):
                                ps = pskp.tile([128, 512], f32, name="psk")
                                for fc in range(FC):
                                    nc.tensor.matmul(
                                        ps[:, :],
                                        wq[fc][:, m * 128:(m + 1) * 128],
                                        xt(fc, 0)[:, :],
                                        start=(fc == 0), stop=(fc == FC - 1))
                                nc.vector.tensor_copy(qT[m][:, :], ps[:, :])

                            # ---- B: EB = exp(biasT + maskneg); biasT loads
                            # ride the ACT hwdge queue so the SP queue's A4
                            # v' stores aren't blocked behind them ----
                            with tc.tile_pool(name="bT", bufs=4) as bTp:
                                for kc in range(KC):
                                    bT = bTp.tile([128, Q], f32, name="bT")
                                    nc.scalar.dma_start(
                                        bT[:, :],
                                        biasT_in[kc * 128:(kc + 1) * 128, :])
                                    nc.scalar.activation(
                                        EB[kc][:, 0:Q], bT[:, :], AF.Exp,
                                        bias=masksb[:, kc:kc + 1], scale=1.0)
                                    nc.vector.tensor_copy(EB[kc][:, Q:2 * Q],
                                                          EB[kc][:, 0:Q])

                            # A4: v' = [x @ Wv | 1] -> DRAM bf16, keys-major
                            with tc.tile_pool(name="vstg", bufs=6) as vstgp:
                                for kc in range(KC):
                                    vst = vstgp.tile([128, H * DV], bf16,
                                                     name="vst")
                                    nc.vector.memset(
                                        vst[:, :].rearrange(
                                            "p (h x) -> p h x",
                                            x=DV)[:, :, D:DV], 1.0)
                                    ps = ps2p.tile([128, 1024], f32,
                                                   name="ps2")
                                    for half in range(2):
                                        for fc in range(FC):
                                            nc.tensor.matmul(
                                                ps[:, half * 512:
                                                   (half + 1) * 512],
                                                xt(fc, kc // 4)[
                                                    :, (kc % 4) * 128:
                                                    (kc % 4 + 1) * 128],
                                                wv[fc][:, half * 512:
                                                       (half + 1) * 512],
                                                start=(fc == 0),
                                                stop=(fc == FC - 1))
                                    nc.vector.tensor_copy(
                                        vst[:, :].rearrange(
                                            "p (h x) -> p h x",
                                            x=DV)[:, :, 0:D],
                                        ps[:, :].rearrange(
                                            "p (h d) -> p h d", d=D))
                                    nc.sync.dma_start(
                                        vprime[kc * 128:(kc + 1) * 128, :],
                                        vst[:, :])

                        # ---- C: attention; k-slab hp+1 on PE under the
                        # hp kc-loop; v' quad loads (520B rows) ----
                        with (
                            tc.tile_pool(name="wkhp", bufs=2) as wkhpp,
                            tc.tile_pool(name="kst", bufs=2) as kstp,
                            tc.tile_pool(name="vph", bufs=2) as vphp,
                            tc.tile_pool(name="eraw", bufs=3) as erawp,
                            tc.tile_pool(name="et", bufs=7) as ep,
                            tc.tile_pool(name="dsb", bufs=2) as dsbp,
                            tc.tile_pool(name="rrep", bufs=2) as rrepp,
                            tc.tile_pool(name="otst", bufs=2) as otstp,
                        ):
                            def load_wk(hp):
                                wkhp = wkhpp.tile([128, F], fr, name="wkhp")
                                nc.sync.dma_start(
                                    wkhp[:, :].rearrange(
                                        "p (fc c) -> p fc c", c=128),
                                    wkv_in[0:F, hp * 128:(hp + 1) * 128]
                                    .rearrange("(fc p) c -> p fc c", p=128))
                                return wkhp

                            def load_vquad(hq):
                                vph = vphp.tile([128, KC * 4 * DV], bf16,
                                                name="vph")
                                nc.sync.dma_start(
                                    vph[:, :].rearrange(
                                        "p (kc d) -> p kc d", d=4 * DV),
                                    vprime[:, 4 * hq * DV:(4 * hq + 4) * DV]
                                    .rearrange("(kc p) d -> p kc d", p=128))
                                return vph

                            def a3_slab(wkhp):
                                """k-slab matmuls as closures, 1 per call."""
                                kst = kstp.tile([128, NK], fr, name="kst")
                                state = {}

                                def mk(nb, fc):
                                    def run():
                                        if fc == 0:
                                            state[nb] = pskp.tile(
                                                [128, 512], f32, name="psk")
                                        nc.tensor.matmul(
                                            state[nb][:, :],
                                            wkhp[:, fc * 128:(fc + 1) * 128],
                                            xt(fc, nb)[:, :],
                                            start=(fc == 0),
                                            stop=(fc == FC - 1))
                                        if fc == FC - 1:
                                            nc.vector.tensor_copy(
                                                kst[:, nb * 512:
                                                    (nb + 1) * 512],
                                                state.pop(nb)[:, :])
                                    return run

                                return kst, [mk(nb, fc) for nb in range(NB)
                                             for fc in range(FC)]

                            def emit_denoms(hp0, psU0):
                                """Softmax normalize + oM write for pair hp0;
                                deferred into the next pair's kc loop so the
                                PE sim stream isn't interrupted."""
                                for sub in range(2):
                                    Dsb = dsbp.tile([DV, 512], fr,
                                                    name="Dsb")
                                    nc.vector.tensor_copy(
                                        Dsb[64:65, :], psU0[sub][64:65, :])
                                    psR = pskp.tile([128, 512], f32,
                                                    name="psk")
                                    nc.tensor.matmul(psR[0:64, :],
                                                     ones_r[64:65, 0:64],
                                                     Dsb[64:65, :],
                                                     start=True, stop=True)
                                    rrep = rrepp.tile([64, 512], f32,
                                                      name="rrep")
                                    nc.vector.reciprocal_approx_fast(
                                        out=rrep[:, :], in_=psR[0:64, :])
                                    ot = otstp.tile([64, Q], bf16, name="ot")
                                    nc.vector.tensor_mul(ot[:, :],
                                                         psU0[sub][0:64, :],
                                                         rrep[:, :])
                                    nc.sync.dma_start(
                                        oM[hp0][sub * 64:(sub + 1) * 64, :],
                                        ot[:, :])

                            wk0 = load_wk(0)
                            kst_cur, ops0 = a3_slab(wk0)
                            for op in ops0:
                                op()
                            wk_next = load_wk(1)
                            vph_cur = load_vquad(0)
                            prev = None

                            for hp in range(HP):
                                if hp == 6:
                                    for i in range(H // 2):
                                        nc.sync.dma_start(
                                            wo[i][:, :],
                                            wo_in[i * 128:(i + 1) * 128, :])
                                if hp % 2 == 1 and hp + 1 < HP:
                                    vph_next = load_vquad((hp + 1) // 2)
                                if hp + 1 < HP:
                                    kst_next, a3_ops = a3_slab(wk_next)
                                    if hp + 2 < HP:
                                        wk_next = load_wk(hp + 2)
                                else:
                                    kst_next, a3_ops = None, []
                                psU = [psup.tile([DV, 512], f32, name="psu")
                                       for _ in range(2)]
                                pending = []

                                def drain_av(upto):
                                    while pending and pending[0][0] <= upto:
                                        kc0, eT = pending.pop(0)
                                        for sub in range(2):
                                            hq_off = (2 * hp + sub) % 4
                                            nc.tensor.matmul(
                                                psU[sub][:, :],
                                                vph_cur[:,
                                                        kc0 * 4 * DV
                                                        + hq_off * DV:
                                                        kc0 * 4 * DV
                                                        + (hq_off + 1) * DV],
                                                eT[:, sub * Q:(sub + 1) * Q],
                                                start=(kc0 == 0),
                                                stop=(kc0 == KC - 1))

                                for kc in range(KC):
                                    ps = ps2p.tile([128, 1024], f32,
                                                   name="ps2")
                                    for sub in range(2):
                                        po = sub * 64
                                        nc.tensor.matmul(
                                            ps[:, sub * Q:(sub + 1) * Q],
                                            kst_cur[po:po + 64,
                                                    kc * 128:(kc + 1) * 128],
                                            qT[hp][po:po + 64, :],
                                            start=True, stop=True)
                                    if kc == 1 and prev is not None:
                                        emit_denoms(*prev)
                                    # 2 slab matmuls for hp+1 per kc chunk
                                    for _ in range(2):
                                        if a3_ops:
                                            a3_ops.pop(0)()
                                    eRaw = erawp.tile([128, 1024], bf16,
                                                      name="eRaw")
                                    nc.scalar.activation(
                                        eRaw[:, :], ps[:, :], AF.Exp,
                                        scale=SCALE)
                                    eT = ep.tile([128, 1024], bf16,
                                                 name="eT")
                                    # every 3rd multiply rides the otherwise-
                                    # idle GpSimd engine to unload DVE
                                    eng = (nc.gpsimd if kc % 3 == 0
                                           else nc.vector)
                                    eng.tensor_mul(eT[:, :], eRaw[:, :],
                                                   EB[kc][:, :])
                                    pending.append((kc, eT))
                                    drain_av(kc - AV_LAG)
                                for op in a3_ops:
                                    op()
                                drain_av(KC)
                                prev = (hp, psU)
                                kst_cur = kst_next
                                if hp % 2 == 1 and hp + 1 < HP:
                                    vph_cur = vph_next
                            emit_denoms(*prev)

                # ======== stage D ========
                with (
                    tc.tile_pool(name="fin", bufs=3) as finp,
                    tc.tile_pool(name="bop", bufs=1) as bop,
                ):
                    bo_sb = bop.tile([1, F], fr, name="bo_sb")
                    nc.sync.dma_start(bo_sb[:, :], bo_in[:, :])
                    bo_rep = bop.tile([128, F], f32, name="bo_rep")
                    for nb2 in range(2):
                        ps = pskp.tile([128, 512], f32, name="psk")
                        nc.tensor.matmul(ps[:, :], ones_r[0:1, 0:128],
                                         bo_sb[0:1, nb2 * 512:(nb2 + 1) * 512],
                                         start=True, stop=True)
                        nc.vector.tensor_copy(
                            bo_rep[:, nb2 * 512:(nb2 + 1) * 512], ps[:, :])
                    for mc in range(4):
                        for nb2 in range(2):
                            psF = pskp.tile([128, 512], f32, name="psk")
                            for h in range(H // 2):
                                nc.tensor.matmul(
                                    psF[:, :],
                                    oM[h][:, mc * 128:(mc + 1) * 128],
                                    wo[h][:, nb2 * 512:(nb2 + 1) * 512],
                                    start=(h == 0), stop=(h == H // 2 - 1))
                            fin = finp.tile([128, 512], f32, name="fin")
                            nc.vector.tensor_add(
                                fin[:, :], psF[:, :],
                                bo_rep[:, nb2 * 512:(nb2 + 1) * 512])
                            nc.sync.dma_start(
                                out_t[mc * 128:(mc + 1) * 128,
                                      nb2 * 512:(nb2 + 1) * 512],
                                fin[:, :])

            if niter == 1:
                body()
            else:
                with tc.For_i(0, niter, 1) as iv:
                    body(iv)

    nc.finalize()
    return nc


_nc_cache = {}


def _get_nc(niter=1):
    if niter not in _nc_cache:
        _nc_cache[niter] = build_nc(niter)
    return _nc_cache[niter]


def make_in_maps(x, bias, mask, Wq, Wkv, Wo, bo):
    x = np.asarray(x, dtype=np.float32)
    bias = np.asarray(bias, dtype=np.float32)
    mask = np.asarray(mask)
    in_maps = []
    for c in range(8):
        b, qi = c // 4, c % 4
        q0 = qi * Q
        # roll the KEY axis by -q0 (softmax/attention are permutation-
        # invariant over keys when bias/mask roll consistently); the core's
        # query block is then always xT columns 0:512
        maskneg = np.roll(
            np.where(mask[b], 0.0, MASK_NEG).astype(np.float32), -q0)
        in_maps.append({
            "xT_in": np.ascontiguousarray(np.roll(x[b].T, -q0, axis=1)),
            "biasT_in": np.ascontiguousarray(
                np.roll(bias[b, q0:q0 + Q].T, -q0, axis=0)),
            "maskneg_in": np.ascontiguousarray(maskneg.reshape(KC, 128).T),
            "wq_in": np.ascontiguousarray(np.asarray(Wq, dtype=np.float32)),
            "wkv_in": np.ascontiguousarray(np.asarray(Wkv, dtype=np.float32)),
            "wo_in": np.ascontiguousarray(
                np.asarray(Wo).astype(mybir.dt.np(bf16))),
            "bo_in": np.ascontiguousarray(
                np.asarray(bo, dtype=np.float32).reshape(1, F)),
        })
    return in_maps


class _CachedRunner:
    """Jit the NEFF-backed executable once; repeat kernel() calls then skip
    the ~40s relower/recompile and run in ~0.1s."""

    def __init__(self, nc, n_cores=8):
        import jax
        from jax.sharding import Mesh, PartitionSpec
        from jax.experimental.shard_map import shard_map
        from concourse.bass2jax import (_bass_exec_p, install_neuronx_cc_hook,
                                        partition_id_tensor)
        install_neuronx_cc_hook()
        self.jax = jax
        self.n_cores = n_cores
        pname = nc.partition_id_tensor.name if nc.partition_id_tensor else None
        in_names, out_names, out_avals, zeros = [], [], [], []
        for alloc in nc.m.functions[0].allocations:
            if not isinstance(alloc, mybir.MemoryLocationSet):
                continue
            name = alloc.memorylocations[0].name
            if alloc.kind == "ExternalInput":
                if name != pname:
                    in_names.append(name)
            elif alloc.kind == "ExternalOutput":
                out_names.append(name)
                shape = tuple(alloc.tensor_shape)
                dt_np = mybir.dt.np(alloc.dtype)
                out_avals.append(jax.core.ShapedArray(shape, dt_np))
                zeros.append(np.zeros(shape, dt_np))
        self.in_names, self.out_names = in_names, out_names
        self.out_avals, self.zeros = out_avals, zeros
        all_names = in_names + out_names + ([pname] if pname else [])

        def _body(*args):
            ops = list(args)
            if pname is not None:
                ops.append(partition_id_tensor())
            return tuple(_bass_exec_p.bind(
                *ops, out_avals=tuple(out_avals), in_names=tuple(all_names),
                out_names=tuple(out_names), lowering_input_output_aliases=(),
                sim_require_finite=True, sim_require_nnan=True, nc=nc))

        mesh = Mesh(np.asarray(jax.devices()[:n_cores]), ("core",))
        spec_in = (PartitionSpec("core"),) * (len(in_names) + len(out_names))
        spec_out = (PartitionSpec("core"),) * len(out_names)
        self.fn = jax.jit(shard_map(_body, mesh=mesh, in_specs=spec_in,
                                    out_specs=spec_out, check_rep=False),
                          keep_unused=True)

    def run(self, in_maps):
        n = self.n_cores
        args = [np.concatenate([np.asarray(in_maps[c][k]) for c in range(n)], axis=0)
                for k in self.in_names]
        args += [np.zeros((n * z.shape[0], *z.shape[1:]), z.dtype)
                 for z in self.zeros]
        outs = self.fn(*args)
        self.jax.block_until_ready(outs)
        return [{k: np.asarray(outs[i]).reshape(n, *self.out_avals[i].shape)[c]
                 for i, k in enumerate(self.out_names)} for c in range(n)]


_runner_cache = {}


def kernel(x, bias, mask, Wq, Wkv, Wo, bo):
    in_maps = make_in_maps(x, bias, mask, Wq, Wkv, Wo, bo)
    try:
        if "r" not in _runner_cache:
            _runner_cache["r"] = _CachedRunner(_get_nc(1))
        results = _runner_cache["r"].run(in_maps)
    except Exception:
        _runner_cache.pop("r", None)
        res = run_bass_kernel_spmd(_get_nc(1), in_maps, core_ids=list(range(8)))
        results = res.results
    out = np.empty((2, NK, F), dtype=np.float32)
    for c in range(8):
        b, qi = c // 4, c % 4
        out[b, qi * Q:(qi + 1) * Q] = results[c]["out_t"]
    return out


# revision 32
# speedup vs baseline: 1.5218x; 1.5179x over previous
"""Trainium2 Bass kernel for nn_Attention_88441966559243.

Attention with additive bias [B,N,N] and per-key bool mask, fp32.
  B=2, N=2048, QD=1024, HEADS=16, DIM_HEAD=64.

Sharding: 8 cores = (batch b = core//4) x (query slice q0 = (core%4)*512).
Each core computes out[b, q0:q0+512, :] completely on-device; the host gather
is a pure concatenation. No collectives.

Design (vs the PE-transpose + 4x-ident-inject baseline at ~750us):
  - x and the core's bias slice are uploaded pre-transposed, with the KEY
    axis rolled by -q0 per core (host layout changes only): no PE transposes
    on device, and the query block is always xT columns 0:512 so no separate
    xq input is needed.  Attention is permutation-invariant over keys since
    bias/mask are rolled consistently.
  - EB = exp(biasT + maskneg) is precomputed once per core on ACT (exp(-30000)
    underflows to exactly 0, folding the key mask in); the softmax numerator
    is then e = exp(sim*scale) * EB, bf16, with the multiply on DVE (every
    3rd on GpSimd).  This removes the old per-tile PE identity-matmul bias
    injection (~220us of PE) entirely; sim*scale rides the ACT scale operand.
  - k^T slabs stay SBUF-resident: each head-pair's slab is computed on PE
    interleaved with the previous pair's attention kc-loop, so there is no
    k DRAM round-trip and the slab matmuls hide under the ACT exp stream
    (the critical path, ~1us per 128x1024 two-head exp tile).  Paired-head
    sim matmuls (K=64) land on PE row-groups 0/64 and run concurrently.
  - v' = [v | 1] in bf16 goes through DRAM once (quad-head-packed loads,
    520B rows); ACT does only exp; PSUM evacuations ride on DVE; softmax
    denominators come free via the ones column + a PE row-replicate, with
    the normalize deferred into the next pair's kc loop.
  - Output projection keeps ot in SBUF (partition-shifting DMA into
    head-pair oM tiles) and runs D as K=128 bf16 matmuls; biasT loads ride
    the ACT hwdge queue to avoid head-of-line blocking of v' stores.

Measured on HW (8 cores, For_i-loop steady state): ~210-220 us/invocation,
rel err vs fp32 jax reference ~4.4e-3 (bf16 e/v/Wo rounding).
"""
import sys
for _p in ("/opt/trn_rl_repo", "/root/.axon_site/_ro/trn_rl_repo"):
    if _p not in sys.path:
        sys.path.insert(0, _p)

import numpy as np

import concourse.bass as bass
import concourse.mybir as mybir
from concourse import bacc
from concourse.tile import TileContext
from concourse.bass_utils import run_bass_kernel_spmd

F = 1024          # feature dim (QD == INNER)
NK = 2048         # keys (full sequence)
Q = 512           # queries per core
H = 16            # heads
D = 64            # head dim
DV = 65           # head dim + ones column
SCALE = D ** -0.5
MASK_NEG = -30000.0

FC = F // 128      # 8 feature chunks
KC = NK // 128     # 16 key chunks
HP = H // 2        # 8 head pairs
NB = NK // 512     # 4 key 512-blocks

f32 = mybir.dt.float32
fr = mybir.dt.float32r
bf16 = mybir.dt.bfloat16
AF = mybir.ActivationFunctionType

AV_LAG = 4         # av matmuls trail the exp/mul producers by this many kc


def build_nc(niter: int = 1):
    nc = bacc.Bacc(None, target_bir_lowering=False)

    xT_in = nc.dram_tensor("xT_in", [F, NK], fr, kind="ExternalInput")
    biasT_in = nc.dram_tensor("biasT_in", [NK, Q], f32, kind="ExternalInput")
    maskneg_in = nc.dram_tensor("maskneg_in", [128, KC], f32, kind="ExternalInput")
    wq_in = nc.dram_tensor("wq_in", [F, F], fr, kind="ExternalInput")
    wkv_in = nc.dram_tensor("wkv_in", [F, 2 * F], fr, kind="ExternalInput")
    wo_in = nc.dram_tensor("wo_in", [F, F], bf16, kind="ExternalInput")
    bo_in = nc.dram_tensor("bo_in", [1, F], fr, kind="ExternalInput")
    out_t = nc.dram_tensor("out_t", [Q, F], f32, kind="ExternalOutput")

    with TileContext(nc) as tc:
        with (
            tc.tile_pool(name="const", bufs=1) as constp,
            tc.tile_pool(name="dram", bufs=1, space="DRAM") as dramp,
            tc.tile_pool(name="ps2", bufs=2, space="PSUM") as ps2p,   # [128,1024]
            tc.tile_pool(name="psk", bufs=2, space="PSUM") as pskp,   # [128,512]
            tc.tile_pool(name="psu", bufs=2, space="PSUM") as psup,   # [DV,512]
        ):
            ones_f = constp.tile([128, 128], f32)
            nc.vector.memset(ones_f[:, :], 1.0)
            ones_r = constp.tile([128, 128], fr)
            nc.scalar.copy(ones_r[:, :], ones_f[:, :])
            masksb = constp.tile([128, KC], f32)
            nc.sync.dma_start(masksb[:, :], maskneg_in[:, :])

            vprime = dramp.tile([NK, H * DV], bf16)     # v' (keys-major)

            def body(_iv=None):
              with (
                  tc.tile_pool(name="oMp", bufs=1) as oMp,
                  tc.tile_pool(name="wop", bufs=1) as wop,
              ):
                # out^T head-pair tiles (bf16), written in-place by C's ot
                # DMAs; wo (bf16) loaded near the end of C.  Stacking heads
                # 2i/2i+1 on partitions 0:64/64:128 makes D K=128 full-rate.
                oM = [oMp.tile([128, Q], bf16, tag=f"oM{i}", name=f"oM{i}")
                      for i in range(H // 2)]
                wo = [wop.tile([128, F], bf16, tag=f"wo{i}", name=f"wo{i}")
                      for i in range(H // 2)]
                with (
                    tc.tile_pool(name="EBp", bufs=1) as EBp,
                    tc.tile_pool(name="qTp", bufs=1) as qTp,
                ):
                    EB = [EBp.tile([128, 2 * Q], bf16, tag=f"EB{i}",
                                   name=f"EB{i}") for i in range(KC)]
                    qT = [qTp.tile([128, Q], fr, tag=f"qT{i}", name=f"qT{i}")
                          for i in range(FC)]

                    with tc.tile_pool(name="xTp", bufs=1) as xTp:
                        # xT as 32 column tiles, loaded col-block-major AFTER
                        # xqT/wq/wv so A2 starts at ~6MB and A4 streams.
                        xTc = [xTp.tile([128, 512], fr, tag=f"xT{i}",
                                        name=f"xT{i}") for i in range(4 * FC)]

                        def xt(fc, cb):
                            return xTc[fc * 4 + cb]

                        # ---- A: q and v' projections (weights scoped) ----
                        # host rolls the key axis by -q0 per core, so the
                        # query block is always xT columns 0:512 (cb 0)
                        with tc.tile_pool(name="wload", bufs=8) as wlp:
                            wq = [wlp.tile([128, F], fr, tag="w", name="w")
                                  for _ in range(FC)]
                            for fc in range(FC):
                                nc.sync.dma_start(
                                    wq[fc][:, :],
                                    wq_in[fc * 128:(fc + 1) * 128, :])
                            for fc in range(FC):
                                nc.sync.dma_start(
                                    xt(fc, 0)[:, :],
                                    xT_in[fc * 128:(fc + 1) * 128, 0:512])
                            wv = [wlp.tile([128, F], fr, tag="w", name="w")
                                  for _ in range(FC)]
                            for fc in range(FC):
                                nc.sync.dma_start(
                                    wv[fc][:, :],
                                    wkv_in[fc * 128:(fc + 1) * 128, F:2 * F])
                            for cb in range(1, 4):
                                for fc in range(FC):
                                    nc.sync.dma_start(
                                        xt(fc, cb)[:, :],
                                        xT_in[fc * 128:(fc + 1) * 128,
                                              cb * 512:(cb + 1) * 512])

                            # A2: qT[m] = Wq[:,m]^T @ xT[:, 0:Q]  (unscaled;
                            # SCALE rides on the exp activation's scale)
                            for m in range(FC):
                                ps = pskp.tile([128, 512], f32, name="psk")
                                for fc in range(FC):
                                    nc.tensor.matmul(
                                        ps[:, :],
                                        wq[fc][:, m * 128:(m + 1) * 128],
                                        xt(fc, 0)[:, :],
                                        start=(fc == 0), stop=(fc == FC - 1))
                                nc.vector.tensor_copy(qT[m][:, :], ps[:, :])

                            # ---- B: EB = exp(biasT + maskneg); biasT loads
                            # ride the ACT hwdge queue so the SP queue's A4
                            # v' stores aren't blocked behind them ----
                            with tc.tile_pool(name="bT", bufs=4) as bTp:
                                for kc in range(KC):
                                    bT = bTp.tile([128, Q], f32, name="bT")
                                    nc.scalar.dma_start(
                                        bT[:, :],
                                        biasT_in[kc * 128:(kc + 1) * 128, :])
                                    nc.scalar.activation(
                                        EB[kc][:, 0:Q], bT[:, :], AF.Exp,
                                        bias=masksb[:, kc:kc + 1], scale=1.0)
                                    nc.vector.tensor_copy(EB[kc][:, Q:2 * Q],
                                                          EB[kc][:, 0:Q])

                            # A4: v' = [x @ Wv | 1] -> DRAM bf16, keys-major
                            with tc.tile_pool(name="vstg", bufs=6) as vstgp:
                                for kc in range(KC):
                                    vst = vstgp.tile([128, H * DV], bf16,
                                                     name="vst")
                                    nc.vector.memset(
                                        vst[:, :].rearrange(
                                            "p (h x) -> p h x",
                                            x=DV)[:, :, D:DV], 1.0)
                                    ps = ps2p.tile([128, 1024], f32,
                                                   name="ps2")
                                    for half in range(2):
                                        for fc in range(FC):
                                            nc.tensor.matmul(
                                                ps[:, half * 512:
                                                   (half + 1) * 512],
                                                xt(fc, kc // 4)[
                                                    :, (kc % 4) * 128:
                                                    (kc % 4 + 1) * 128],
                                                wv[fc][:, half * 512:
                                                       (half + 1) * 512],
                                                start=(fc == 0),
                                                stop=(fc == FC - 1))
                                    nc.vector.tensor_copy(
                                        vst[:, :].rearrange(
                                            "p (h x) -> p h x",
                                            x=DV)[:, :, 0:D],
                                        ps[:, :].rearrange(
                                            "p (h d) -> p h d", d=D))
                                    nc.sync.dma_start(
                                        vprime[kc * 128:(kc + 1) * 128, :],
                                        vst[:, :])

                        # ---- C: attention; k-slab hp+1 on PE under the
                        # hp kc-loop; v' quad loads (520B rows) ----
                        with (
                            tc.tile_pool(name="wkhp", bufs=2) as wkhpp,
                            tc.tile_pool(name="kst", bufs=2) as kstp,
                            tc.tile_pool(name="vph", bufs=2) as vphp,
                            tc.tile_pool(name="eraw", bufs=3) as erawp,
                            tc.tile_pool(name="et", bufs=7) as ep,
                            tc.tile_pool(name="dsb", bufs=2) as dsbp,
                            tc.tile_pool(name="rrep", bufs=2) as rrepp,
                            tc.tile_pool(name="otst", bufs=2) as otstp,
                        ):
                            def load_wk(hp):
                                wkhp = wkhpp.tile([128, F], fr, name="wkhp")
                                nc.sync.dma_start(
                                    wkhp[:, :].rearrange(
                                        "p (fc c) -> p fc c", c=128),
                                    wkv_in[0:F, hp * 128:(hp + 1) * 128]
                                    .rearrange("(fc p) c -> p fc c", p=128))
                                return wkhp

                            def load_vquad(hq):
                                vph = vphp.tile([128, KC * 4 * DV], bf16,
                                                name="vph")
                                nc.sync.dma_start(
                                    vph[:, :].rearrange(
                                        "p (kc d) -> p kc d", d=4 * DV),
                                    vprime[:, 4 * hq * DV:(4 * hq + 4) * DV]
                                    .rearrange("(kc p) d -> p kc d", p=128))
                                return vph

                            def a3_slab(wkhp):
                                """k-slab matmuls as closures, 1 per call."""
                                kst = kstp.tile([128, NK], fr, name="kst")
                                state = {}

                                def mk(nb, fc):
                                    def run():
                                        if fc == 0:
                                            state[nb] = pskp.tile(
                                                [128, 512], f32, name="psk")
                                        nc.tensor.matmul(
                                            state[nb][:, :],
                                            wkhp[:, fc * 128:(fc + 1) * 128],
                                            xt(fc, nb)[:, :],
                                            start=(fc == 0),
                                            stop=(fc == FC - 1))
                                        if fc == FC - 1:
                                            nc.vector.tensor_copy(
                                                kst[:, nb * 512:
                                                    (nb + 1) * 512],
                                                state.pop(nb)[:, :])
                                    return run

                                return kst, [mk(nb, fc) for nb in range(NB)
                                             for fc in range(FC)]

                            def emit_denoms(hp0, psU0):
                                """Softmax normalize + oM write for pair hp0;
                                deferred into the next pair's kc loop so the
                                PE sim stream isn't interrupted."""
                                for sub in range(2):
                                    Dsb = dsbp.tile([DV, 512], fr,
                                                    name="Dsb")
                                    nc.vector.tensor_copy(
                                        Dsb[64:65, :], psU0[sub][64:65, :])
                                    psR = pskp.tile([128, 512], f32,
                                                    name="psk")
                                    nc.tensor.matmul(psR[0:64, :],
                                                     ones_r[64:65, 0:64],
                                                     Dsb[64:65, :],
                                                     start=True, stop=True)
                                    rrep = rrepp.tile([64, 512], f32,
                                                      name="rrep")
                                    nc.vector.reciprocal_approx_fast(
                                        out=rrep[:, :], in_=psR[0:64, :])
                                    ot = otstp.tile([64, Q], bf16, name="ot")
                                    nc.vector.tensor_mul(ot[:, :],
                                                         psU0[sub][0:64, :],
                                                         rrep[:, :])
                                    nc.sync.dma_start(
                                        oM[hp0][sub * 64:(sub + 1) * 64, :],
                                        ot[:, :])

                            wk0 = load_wk(0)
                            kst_cur, ops0 = a3_slab(wk0)
                            for op in ops0:
                                op()
                            wk_next = load_wk(1)
                            vph_cur = load_vquad(0)
                            prev = None

                            for hp in range(HP):
                                if hp == 6:
                                    for i in range(H // 2):
                                        nc.sync.dma_start(
                                            wo[i][:, :],
                                            wo_in[i * 128:(i + 1) * 128, :])
                                if hp % 2 == 1 and hp + 1 < HP:
                                    vph_next = load_vquad((hp + 1) // 2)
                                if hp + 1 < HP:
                                    kst_next, a3_ops = a3_slab(wk_next)
                                    if hp + 2 < HP:
                                        wk_next = load_wk(hp + 2)
                                else:
                                    kst_next, a3_ops = None, []
                                psU = [psup.tile([DV, 512], f32, name="psu")
                                       for _ in range(2)]
                                pending = []

                                def drain_av(upto):
                                    while pending and pending[0][0] <= upto:
                                        kc0, eT = pending.pop(0)
                                        for sub in range(2):
                                            hq_off = (2 * hp + sub) % 4
                                            nc.tensor.matmul(
                                                psU[sub][:, :],
                                                vph_cur[:,
                                                        kc0 * 4 * DV
                                                        + hq_off * DV:
                                                        kc0 * 4 * DV
                                                        + (hq_off + 1) * DV],
                                                eT[:, sub * Q:(sub + 1) * Q],
                                                start=(kc0 == 0),
                                                stop=(kc0 == KC - 1))

                                for kc in range(KC):
                                    ps = ps2p.tile([128, 1024], f32,
                                                   name="ps2")
                                    for sub in range(2):
                                        po = sub * 64
                                        nc.tensor.matmul(
                                            ps[:, sub * Q:(sub + 1) * Q],
                                            kst_cur[po:po + 64,
                                                    kc * 128:(kc + 1) * 128],
                                            qT[hp][po:po + 64, :],
                                            start=True, stop=True)
                                    if kc == 1 and prev is not None:
                                        emit_denoms(*prev)
                                    # 2 slab matmuls for hp+1 per kc chunk
                                    for _ in range(2):
                                        if a3_ops:
                                            a3_ops.pop(0)()
                                    eRaw = erawp.tile([128, 1024], bf16,
                                                      name="eRaw")
                                    nc.scalar.activation(
                                        eRaw[:, :], ps[:, :], AF.Exp,
                                        scale=SCALE)
                                    eT = ep.tile([128, 1024], bf16,
                                                 name="eT")
                                    # every 3rd multiply rides the otherwise-
                                    # idle GpSimd engine to unload DVE
                                    eng = (nc.gpsimd if kc % 3 == 0
                                           else nc.vector)
                                    eng.tensor_mul(eT[:, :], eRaw[:, :],
                                                   EB[kc][:, :])
                                    pending.append((kc, eT))
                                    drain_av(kc - AV_LAG)
                                for op in a3_ops:
                                    op()
                                drain_av(KC)
                                prev = (hp, psU)
                                kst_cur = kst_next
                                if hp % 2 == 1 and hp + 1 < HP:
                                    vph_cur = vph_next
                            emit_denoms(*prev)

                # ======== stage D ========
                with (
                    tc.tile_pool(name="fin", bufs=3) as finp,
                    tc.tile_pool(name="bop", bufs=1) as bop,
                ):
                    bo_sb = bop.tile([1, F], fr, name="bo_sb")
                    nc.sync.dma_start(bo_sb[:, :], bo_in[:, :])
                    bo_rep = bop.tile([128, F], f32, name="bo_rep")
                    for nb2 in range(2):
                        ps = pskp.tile([128, 512], f32, name="psk")
                        nc.tensor.matmul(ps[:, :], ones_r[0:1, 0:128],
                                         bo_sb[0:1, nb2 * 512:(nb2 + 1) * 512],
                                         start=True, stop=True)
                        nc.vector.tensor_copy(
                            bo_rep[:, nb2 * 512:(nb2 + 1) * 512], ps[:, :])
                    for mc in range(4):
                        for nb2 in range(2):
                            psF = pskp.tile([128, 512], f32, name="psk")
                            for h in range(H // 2):
                                nc.tensor.matmul(
                                    psF[:, :],
                                    oM[h][:, mc * 128:(mc + 1) * 128],
                                    wo[h][:, nb2 * 512:(nb2 + 1) * 512],
                                    start=(h == 0), stop=(h == H // 2 - 1))
                            fin = finp.tile([128, 512], f32, name="fin")
                            nc.vector.tensor_add(
                                fin[:, :], psF[:, :],
                                bo_rep[:, nb2 * 512:(nb2 + 1) * 512])
                            nc.sync.dma_start(
                                out_t[mc * 128:(mc + 1) * 128,
                                      nb2 * 512:(nb2 + 1) * 512],
                                fin[:, :])

            if niter == 1:
                body()
            else:
                with tc.For_i(0, niter, 1) as iv:
                    body(iv)

    nc.finalize()
    return nc


_nc_cache = {}


def _get_nc(niter=1):
    if niter not in _nc_cache:
        _nc_cache[niter] = build_nc(niter)
    return _nc_cache[niter]


def make_in_maps(x, bias, mask, Wq, Wkv, Wo, bo):
    x = np.asarray(x, dtype=np.float32)
    bias = np.asarray(bias, dtype=np.float32)
    mask = np.asarray(mask)
    in_maps = []
    for c in range(8):
        b, qi = c // 4, c % 4
        q0 = qi * Q
        # roll the KEY axis by -q0 (softmax/attention are permutation-
        # invariant over keys when bias/mask roll consistently); the core's
        # query block is then always xT columns 0:512
        maskneg = np.roll(
            np.where(mask[b], 0.0, MASK_NEG).astype(np.float32), -q0)
        in_maps.append({
            "xT_in": np.ascontiguousarray(np.roll(x[b].T, -q0, axis=1)),
            "biasT_in": np.ascontiguousarray(
                np.roll(bias[b, q0:q0 + Q].T, -q0, axis=0)),
            "maskneg_in": np.ascontiguousarray(maskneg.reshape(KC, 128).T),
            "wq_in": np.ascontiguousarray(np.asarray(Wq, dtype=np.float32)),
            "wkv_in": np.ascontiguousarray(np.asarray(Wkv, dtype=np.float32)),
            "wo_in": np.ascontiguousarray(
                np.asarray(Wo).astype(mybir.dt.np(bf16))),
            "bo_in": np.ascontiguousarray(
                np.asarray(bo, dtype=np.float32).reshape(1, F)),
        })
    return in_maps


class _CachedRunner:
    """Jit the NEFF-backed executable once; repeat kernel() calls then skip
    the ~40s relower/recompile and run in ~0.1s."""

    def __init__(self, nc, n_cores=8):
        import jax
        from jax.sharding import Mesh, PartitionSpec
        from jax.experimental.shard_map import shard_map
        from concourse.bass2jax import (_bass_exec_p, install_neuronx_cc_hook,
                                        partition_id_tensor)
        install_neuronx_cc_hook()
        self.jax = jax
        self.n_cores = n_cores
        pname = nc.partition_id_tensor.name if nc.partition_id_tensor else None
        in_names, out_names, out_avals, zeros = [], [], [], []
        for alloc in nc.m.functions[0].allocations:
            if not isinstance(alloc, mybir.MemoryLocationSet):
                continue
            name = alloc.memorylocations[0].name
            if alloc.kind == "ExternalInput":
                if name != pname:
                    in_names.append(name)
            elif alloc.kind == "ExternalOutput":
                out_names.append(name)
                shape = tuple(alloc.tensor_shape)
                dt_np = mybir.dt.np(alloc.dtype)
                out_avals.append(jax.core.ShapedArray(shape, dt_np))
                zeros.append(np.zeros(shape, dt_np))
        self.in_names, self.out_names = in_names, out_names
        self.out_avals, self.zeros = out_avals, zeros
        all_names = in_names + out_names + ([pname] if pname else [])

        def _body(*args):
            ops = list(args)
            if pname is not None:
                ops.append(partition_id_tensor())
            return tuple(_bass_exec_p.bind(
                *ops, out_avals=tuple(out_avals), in_names=tuple(all_names),
                out_names=tuple(out_names), lowering_input_output_aliases=(),
                sim_require_finite=True, sim_require_nnan=True, nc=nc))

        mesh = Mesh(np.asarray(jax.devices()[:n_cores]), ("core",))
        spec_in = (PartitionSpec("core"),) * (len(in_names) + len(out_names))
        spec_out = (PartitionSpec("core"),) * len(out_names)
        self.fn = jax.jit(shard_map(_body, mesh=mesh, in_specs=spec_in,
                                    out_specs=spec_out, check_rep=False),
                          keep_unused=True)

    def run(self, in_maps):
        n = self.n_cores
        args = [np.concatenate([np.asarray(in_maps[c][k]) for c in range(n)], axis=0)
                for k in self.in_names]
        args += [np.zeros((n * z.shape[0], *z.shape[1:]), z.dtype)
                 for z in self.zeros]
        outs = self.fn(*args)
        self.jax.block_until_ready(outs)
        return [{k: np.asarray(outs[i]).reshape(n, *self.out_avals[i].shape)[c]
                 for i, k in enumerate(self.out_names)} for c in range(n)]


_runner_cache = {}


def kernel(x, bias, mask, Wq, Wkv, Wo, bo):
    in_maps = make_in_maps(x, bias, mask, Wq, Wkv, Wo, bo)
    try:
        if "r" not in _runner_cache:
            _runner_cache["r"] = _CachedRunner(_get_nc(1))
        results = _runner_cache["r"].run(in_maps)
    except Exception:
        _runner_cache.pop("r", None)
        res = run_bass_kernel_spmd(_get_nc(1), in_maps, core_ids=list(range(8)))
        results = res.results
    out = np.empty((2, NK, F), dtype=np.float32)
    for c in range(8):
        b, qi = c // 4, c % 4
        out[b, qi * Q:(qi + 1) * Q] = results[c]["out_t"]
    return out
